# revision 1
# baseline (speedup 1.0000x reference)
"""Trainium2 Bass kernel for ContrastHead (softnn contrastive KNN loss).

Data-parallel over points: 12500 points/core on 8 cores. Host packs a table
row per point (256B = 128 f16 slots): [f16 features(64) | f32 ||f||^2 | f16
label | zero pad]. The 100k-row table is laid out as 4 windows of 32766 real
rows, each prefixed by a zero dummy row, so int16 dma_gather indices address
any row: pass w gathers with local idx (gidx - w*32766 + 1) for in-window
slots and 0 (dummy zero row) otherwise. The four per-window gathers of a
point-tile land in four SBUF buffers merged with bitwise-OR (zero rows are
identity). Per tile: DVE f16 multiply + in-place tree-add + reduce -> dots;
dist^2 = s_i + s_j - 2*dot; posmask via is_equal. Whole-core phase 2 on
(128, 3038): sqrt -> row min -> exp((min-d)/T) -> neg/pos sums -> ratio ->
Ln(+1e-8) -> point mask -> per-partition (128, 2) accumulators. Host sums
the 8x(128,2) outputs and divides.
"""

import numpy as np

import concourse.bacc as bacc
import concourse.bass as bass
import concourse.mybir as mybir
import concourse.tile as tile
from concourse import bass_utils

F16 = mybir.dt.float16
F32 = mybir.dt.float32
I16 = mybir.dt.int16

N = 100000
K = 31
C = 64
ROW = 128                   # f16 slots per table row (256B)
WINR = 32766                # real rows per window
WSTR = WINR + 1             # window stride in table (incl dummy row 0)
NWIN = 4
TABROWS = NWIN * WSTR
NCORES = 8
PTS = N // NCORES           # 12500
TPC = (PTS + 127) // 128    # 98 tiles/core
NIDX = K * 128              # 3968 gather slots per tile
NIB16 = NIDX // 16          # 248
TEMP = 0.1
EPS = 1e-8

_CACHE = {}


def _build():
    nc = bacc.Bacc("TRN2", target_bir_lowering=False, debug=False)

    tabT = nc.dram_tensor("tab", (TABROWS, ROW), F16, kind="ExternalInput")
    selfT = nc.dram_tensor("selftab", (128, TPC, ROW), F16, kind="ExternalInput")
    idxT = nc.dram_tensor("nidx16", (NWIN, TPC, 128, NIB16), I16, kind="ExternalInput")
    validT = nc.dram_tensor("valid", (128, TPC), F32, kind="ExternalInput")
    outT = nc.dram_tensor("out", (128, 2), F32, kind="ExternalOutput")

    with tile.TileContext(nc) as tc:
        with (
            tc.tile_pool(name="res", bufs=1) as res,
            tc.tile_pool(name="gpool", bufs=2) as gpool,
            tc.tile_pool(name="ipool", bufs=2) as ipool,
            tc.tile_pool(name="mpool", bufs=1) as mpool,
            tc.tile_pool(name="p2", bufs=1) as p2,
        ):
            selfsb = res.tile([128, TPC, ROW], F16)
            nc.sync.dma_start(out=selfsb[:], in_=selfT.ap())
            validsb = res.tile([128, TPC], F32)
            nc.sync.dma_start(out=validsb[:], in_=validT.ap())

            dist2 = res.tile([128, TPC, K], F32)
            pm = res.tile([128, TPC, K], F32)

            selff32 = selfsb[:].bitcast(F32)        # (128, TPC, 64)

            for t in range(TPC):
                gs = []
                for w in range(NWIN):
                    iv = ipool.tile([128, NIB16], I16, tag=f"i{w}")
                    nc.sync.dma_start(out=iv[:], in_=idxT.ap()[w, t, :, :])
                    g = gpool.tile([128, K, ROW], F16, tag=f"g{w}")
                    nc.gpsimd.dma_gather(
                        out_ap=g[:],
                        in_ap=tabT.ap()[w * WSTR : (w + 1) * WSTR, :],
                        idxs_ap=iv[:],
                        num_idxs=NIDX,
                        num_idxs_reg=NIDX,
                        elem_size=ROW,
                        single_packet=False,
                    )
                    gs.append(g)
                # OR-merge the four windows (zero dummy rows are identity)
                g0 = gs[0]
                m01 = g0[:, :, 0:68].bitcast(I16)
                nc.vector.tensor_tensor(
                    out=m01, in0=m01, in1=gs[1][:, :, 0:68].bitcast(I16),
                    op=mybir.AluOpType.bitwise_or,
                )
                m23 = gs[2][:, :, 0:68].bitcast(I16)
                nc.vector.tensor_tensor(
                    out=m23, in0=m23, in1=gs[3][:, :, 0:68].bitcast(I16),
                    op=mybir.AluOpType.bitwise_or,
                )
                nc.vector.tensor_tensor(
                    out=m01, in0=m01, in1=m23, op=mybir.AluOpType.bitwise_or,
                )

                m = mpool.tile([128, K, C], F16, tag="m")
                fb = selfsb[:, t, 0:C].unsqueeze(1).broadcast_to([128, K, C])
                nc.vector.tensor_tensor(
                    out=m[:], in0=g0[:, :, 0:C], in1=fb, op=mybir.AluOpType.mult
                )
                nc.vector.tensor_add(
                    out=m[:, :, 0:32], in0=m[:, :, 0:32], in1=m[:, :, 32:64]
                )
                nc.vector.tensor_add(
                    out=m[:, :, 0:16], in0=m[:, :, 0:16], in1=m[:, :, 16:32]
                )
                nc.vector.tensor_add(
                    out=m[:, :, 0:8], in0=m[:, :, 0:8], in1=m[:, :, 8:16]
                )
                dslice = dist2[:, t, :]              # (128, K)
                nc.vector.reduce_sum(
                    out=dslice, in_=m[:, :, 0:8], axis=mybir.AxisListType.X
                )
                gf32 = g0[:].bitcast(F32)            # (128, K, 64)
                sj = gf32[:, :, 32]                  # (128, K)
                nc.vector.scalar_tensor_tensor(
                    out=dslice, in0=dslice, scalar=-2.0, in1=sj,
                    op0=mybir.AluOpType.mult, op1=mybir.AluOpType.add,
                )
                si = selff32[:, t, 32].unsqueeze(1).broadcast_to([128, K])
                nc.vector.tensor_add(out=dslice, in0=dslice, in1=si)
                nl = g0[:, :, 66]                    # (128, K)
                li = selfsb[:, t, 66].unsqueeze(1).broadcast_to([128, K])
                nc.vector.tensor_tensor(
                    out=pm[:, t, :], in0=nl, in1=li, op=mybir.AluOpType.is_equal,
                )

            # ---- phase 2 ----
            nc.scalar.sqrt(out=dist2[:], in_=dist2[:])
            mind = p2.tile([128, TPC], F32)
            nc.vector.tensor_reduce(
                out=mind[:], in_=dist2[:], axis=mybir.AxisListType.X,
                op=mybir.AluOpType.min,
            )
            mbc = mind[:].unsqueeze(2).broadcast_to([128, TPC, K])
            nc.vector.tensor_tensor(
                out=dist2[:], in0=dist2[:], in1=mbc, op=mybir.AluOpType.subtract
            )
            nc.scalar.activation(
                out=dist2[:], in_=dist2[:],
                func=mybir.ActivationFunctionType.Exp, scale=-1.0 / TEMP,
            )
            negs = p2.tile([128, TPC], F32)
            nc.vector.reduce_sum(out=negs[:], in_=dist2[:], axis=mybir.AxisListType.X)
            nc.vector.tensor_tensor(
                out=dist2[:], in0=dist2[:], in1=pm[:], op=mybir.AluOpType.mult
            )
            poss = p2.tile([128, TPC], F32)
            nc.vector.reduce_sum(out=poss[:], in_=dist2[:], axis=mybir.AxisListType.X)
            cnts = p2.tile([128, TPC], F32)
            nc.vector.reduce_sum(out=cnts[:], in_=pm[:], axis=mybir.AxisListType.X)
            rn = p2.tile([128, TPC], F32)
            nc.vector.reciprocal(out=rn[:], in_=negs[:])
            ratio = p2.tile([128, TPC], F32)
            nc.vector.tensor_tensor(
                out=ratio[:], in0=poss[:], in1=rn[:], op=mybir.AluOpType.mult
            )
            eps_t = p2.tile([128, 1], F32)
            nc.vector.memset(eps_t[:], EPS)
            lg = p2.tile([128, TPC], F32)
            nc.scalar.activation(
                out=lg[:], in_=ratio[:],
                func=mybir.ActivationFunctionType.Ln, bias=eps_t[:],
            )
            ma = p2.tile([128, TPC], F32)
            nc.vector.tensor_scalar(
                out=ma[:], in0=cnts[:], scalar1=0.5, scalar2=None,
                op0=mybir.AluOpType.is_gt,
            )
            mb2 = p2.tile([128, TPC], F32)
            nc.vector.tensor_scalar(
                out=mb2[:], in0=cnts[:], scalar1=float(K) - 0.5, scalar2=None,
                op0=mybir.AluOpType.is_lt,
            )
            nc.vector.tensor_tensor(
                out=ma[:], in0=ma[:], in1=mb2[:], op=mybir.AluOpType.mult
            )
            nc.vector.tensor_tensor(
                out=ma[:], in0=ma[:], in1=validsb[:], op=mybir.AluOpType.mult
            )
            nc.vector.tensor_tensor(
                out=lg[:], in0=lg[:], in1=ma[:], op=mybir.AluOpType.mult
            )
            outsb = p2.tile([128, 2], F32)
            nc.vector.reduce_sum(out=outsb[:, 0:1], in_=lg[:], axis=mybir.AxisListType.X)
            nc.vector.reduce_sum(out=outsb[:, 1:2], in_=ma[:], axis=mybir.AxisListType.X)
            nc.sync.dma_start(out=outT.ap(), in_=outsb[:])

    nc.compile()
    return nc


def _get_nc():
    if "nc" not in _CACHE:
        _CACHE["nc"] = _build()
    return _CACHE["nc"]


def _pack_table(features: np.ndarray, labels: np.ndarray) -> np.ndarray:
    packed = np.zeros((N, ROW), dtype=np.float16)
    packed[:, 0:C] = features.astype(np.float16)
    s = np.sum(features.astype(np.float64) ** 2, axis=1).astype(np.float32)
    packed[:, 64:66] = s[:, None].view(np.float16)
    packed[:, 66] = labels.astype(np.float16)

    tab = np.zeros((TABROWS, ROW), dtype=np.float16)
    for w in range(NWIN):
        lo = w * WINR
        hi = min(lo + WINR, N)
        if hi > lo:
            tab[w * WSTR + 1 : w * WSTR + 1 + (hi - lo)] = packed[lo:hi]
    return tab


def _core_inputs(table, neighbor_idx, lo, hi):
    npts = hi - lo
    pad = TPC * 128
    # self rows in per-tile partition-major order
    packed_rows = np.zeros((pad, ROW), dtype=np.float16)
    src = neighbor_idx[lo:hi]
    # rebuild self rows from the windowed table
    gidx_self = np.arange(lo, hi)
    w_self = gidx_self // WINR
    self_rows = table[w_self * WSTR + 1 + (gidx_self - w_self * WINR)]
    packed_rows[:npts] = self_rows

    nidx_c = np.zeros((pad, K), dtype=np.int64)
    nidx_c[:npts] = src
    valid_c = np.zeros((pad,), dtype=np.float32)
    valid_c[:npts] = 1.0

    nidx_pm = nidx_c.reshape(TPC, 128, K).transpose(1, 0, 2)   # (128, TPC, K)
    # flat slot order per tile: position i = j*128 + p
    flat = nidx_pm.transpose(1, 2, 0).reshape(TPC, K * 128)    # (TPC, NIDX)
    w_of = flat // WINR
    loc_in = flat - w_of * WINR + 1                            # 1..WINR
    nidx16 = np.zeros((NWIN, TPC, 128, NIB16), dtype=np.int16)
    for w in range(NWIN):
        loc = np.where(w_of == w, loc_in, 0).astype(np.int16)  # (TPC, NIDX)
        wrapped = loc.reshape(TPC, NIB16, 16).transpose(0, 2, 1)  # (TPC, 16, NIB16)
        nidx16[w] = np.tile(wrapped, (1, 8, 1))
    return {
        "tab": table,
        "selftab": np.ascontiguousarray(
            packed_rows.reshape(TPC, 128, ROW).transpose(1, 0, 2)
        ),
        "nidx16": nidx16,
        "valid": np.ascontiguousarray(valid_c.reshape(TPC, 128).transpose(1, 0)),
    }


def run(features, labels, neighbor_idx, trace=False):
    nc = _get_nc()
    table = _pack_table(features, labels)
    in_maps = [
        _core_inputs(table, neighbor_idx, c * PTS, (c + 1) * PTS)
        for c in range(NCORES)
    ]
    res = bass_utils.run_bass_kernel_spmd(
        nc, in_maps, core_ids=list(range(NCORES)), trace=trace
    )
    s = 0.0
    ccnt = 0.0
    for o in res.results:
        s += float(o["out"][:, 0].astype(np.float64).sum())
        ccnt += float(o["out"][:, 1].astype(np.float64).sum())
    loss = np.float32(-s / max(ccnt, 1.0))
    return loss, res


def kernel(features, labels, neighbor_idx):
    loss, _ = run(features, labels, neighbor_idx, trace=False)
    return loss



# revision 3
# speedup vs baseline: 6.1745x; 6.1745x over previous
"""Trainium2 Bass kernel for ContrastHead (softnn contrastive KNN loss).

Data-parallel over points: 12500 points/core on 8 cores. Host packs a table
row per point (256B = 128 f16 slots): [f16 features(64) | f32 ||f||^2 | f16
label | pad]. Table = 4 windows of 32766 rows, each prefixed by a POISON row
(features 0, ||f||^2 = 1e6, label -1) so int16 dma_gather indices address any
row. Each point's 31 neighbors are sorted by window; points are grouped into
128-lane tiles by their window-count profile (lexsort) so that the per-tile
per-window max count is tight. Per tile, one gather per window writes a dense
j-slice of a single (128, JT, 256B) buffer — no dummy-row OR-merge, ~3.1x
fewer DMA descriptors than a fixed 4-window scheme. Ragged slots gather the
poison row, which self-masks: exp((min-1000)/T)=0 and label -1 never equals a
real label. Gathers round-robin the 4 SWDGE queues so descriptor generation
spreads across Q7 core pairs. Per tile: DVE f16 multiply + tree-add + reduce
-> dots; dist^2 = s_i + s_j - 2*dot; posmask via is_equal. Phase 2 on
(128, TPC*JTP): sqrt -> row min -> exp((min-d)/T) -> neg/pos sums -> ratio ->
Ln(+1e-8) -> point mask -> (128, 2) accumulators. Host sums 8x(128,2).
"""

import numpy as np

import concourse.bacc as bacc
import concourse.bass as bass
import concourse.mybir as mybir
import concourse.tile as tile
from concourse import bass_utils

F16 = mybir.dt.float16
F32 = mybir.dt.float32
I16 = mybir.dt.int16

N = 100000
K = 31
C = 64
ROW = 128                   # f16 slots per table row (256B)
WINR = 32766                # real rows per window
WSTR = WINR + 1             # window stride in table (incl poison row 0)
NWIN = 4
TABROWS = NWIN * WSTR
NCORES = 8
PTS = N // NCORES           # 12500
TPC = (PTS + 127) // 128    # 98 tiles/core
TEMP = 0.1
EPS = 1e-8
POIS_S = 1.0e6
NQUEUES = 4

_CACHE = {}


# ---------------- host-side plan + packing ----------------

def _pack_table(features, labels):
    packed = np.zeros((N, ROW), dtype=np.float16)
    packed[:, 0:C] = features.astype(np.float16)
    s = np.sum(features.astype(np.float64) ** 2, axis=1).astype(np.float32)
    packed[:, 64:66] = s[:, None].view(np.float16)
    packed[:, 66] = labels.astype(np.float16)

    tab = np.zeros((TABROWS, ROW), dtype=np.float16)
    pr = np.zeros((ROW,), dtype=np.float16)
    pr[64:66] = np.array([POIS_S], dtype=np.float32).view(np.float16)
    pr[66] = -1.0
    for w in range(NWIN):
        lo = w * WINR
        hi = min(lo + WINR, N)
        tab[w * WSTR] = pr
        if hi > lo:
            tab[w * WSTR + 1 : w * WSTR + 1 + (hi - lo)] = packed[lo:hi]
    return packed, tab


def _build_plan(neighbor_idx):
    """Shared (cross-core) gather plan + per-core neighbor orderings."""
    w_all = (neighbor_idx // WINR).astype(np.int32)
    loc_all = (neighbor_idx - w_all * WINR + 1).astype(np.int32)

    cores = []
    pj = np.zeros((NCORES, TPC, NWIN), np.int32)
    for c in range(NCORES):
        lo, hi = c * PTS, (c + 1) * PTS
        wc = w_all[lo:hi]
        cnt = np.stack([(wc == j).sum(1) for j in range(NWIN)], axis=1).astype(np.int32)
        order = np.lexsort((cnt[:, 2], cnt[:, 1], cnt[:, 0]))
        ks = np.argsort(wc, axis=1, kind="stable")
        loc_s = np.take_along_axis(loc_all[lo:hi], ks, axis=1)
        cum = np.zeros((PTS, NWIN), np.int32)
        cum[:, 1:] = np.cumsum(cnt, axis=1)[:, :-1]
        pad = TPC * 128 - PTS
        cnt_p = np.vstack([cnt[order], np.zeros((pad, NWIN), np.int32)])
        pj[c] = cnt_p.reshape(TPC, 128, NWIN).max(axis=1)
        cores.append(dict(order=order, cnt=cnt, cum=cum, loc_s=loc_s))
    jmax = pj.max(axis=0)                       # (TPC, NWIN)
    joff = np.zeros((TPC, NWIN), np.int32)
    joff[:, 1:] = np.cumsum(jmax, axis=1)[:, :-1]
    JT = jmax.sum(axis=1)                       # (TPC,)
    return dict(jmax=jmax, joff=joff, JT=JT, JTP=int(JT.max()), cores=cores)


def _core_inputs(plan, packed, c):
    info = plan["cores"][c]
    order, cnt, cum, loc_s = info["order"], info["cnt"], info["cum"], info["loc_s"]
    jmax, JT = plan["jmax"], plan["JT"]
    segs = []
    for t in range(TPC):
        base = t * 128
        npts = min(128, PTS - base) if base < PTS else 0
        pts_t = np.zeros(128, np.int64)
        pts_t[:npts] = order[base : base + npts]
        real = np.zeros(128, bool)
        real[:npts] = True
        tile_flat = []
        for w in range(NWIN):
            jm = int(jmax[t, w])
            if jm == 0:
                continue
            n_w = np.where(real, cnt[pts_t, w], 0)
            st = cum[pts_t, w]
            col = st[:, None] + np.arange(jm)[None, :]
            validm = np.arange(jm)[None, :] < n_w[:, None]
            vals = np.where(
                validm,
                np.take_along_axis(loc_s[pts_t], np.minimum(col, K - 1), axis=1),
                0,
            )
            tile_flat.append(vals.T.reshape(-1))     # j-major flat
        flat = np.concatenate(tile_flat).astype(np.int16)
        wrapped = flat.reshape(-1, 16).T             # (16, JT_t*8)
        segs.append(np.tile(wrapped, (8, 1)))        # (128, JT_t*8)
    idx16 = np.ascontiguousarray(np.concatenate(segs, axis=1))

    rows = np.zeros((TPC * 128, ROW), np.float16)
    rows[:PTS] = packed[c * PTS : (c + 1) * PTS][order]
    selftab = np.ascontiguousarray(rows.reshape(TPC, 128, ROW).transpose(1, 0, 2))
    valid = np.zeros(TPC * 128, np.float32)
    valid[:PTS] = 1.0
    valid = np.ascontiguousarray(valid.reshape(TPC, 128).transpose(1, 0))
    return dict(tab=None, selftab=selftab, nidx16=idx16, valid=valid)


# ---------------- device program ----------------

def _build(jmax, joff, JT, JTP, TOTW):
    nc = bacc.Bacc(
        "TRN2", target_bir_lowering=False, debug=False, num_swdge_queues=NQUEUES
    )

    tabT = nc.dram_tensor("tab", (TABROWS, ROW), F16, kind="ExternalInput")
    selfT = nc.dram_tensor("selftab", (128, TPC, ROW), F16, kind="ExternalInput")
    idxT = nc.dram_tensor("nidx16", (128, TOTW), I16, kind="ExternalInput")
    validT = nc.dram_tensor("valid", (128, TPC), F32, kind="ExternalInput")
    outT = nc.dram_tensor("out", (128, 2), F32, kind="ExternalOutput")

    with tile.TileContext(nc) as tc:
        with (
            tc.tile_pool(name="res", bufs=1) as res,
            tc.tile_pool(name="gpool", bufs=2) as gpool,
            tc.tile_pool(name="ipool", bufs=2) as ipool,
            tc.tile_pool(name="mpool", bufs=2) as mpool,
            tc.tile_pool(name="p2", bufs=1) as p2,
        ):
            selfsb = res.tile([128, TPC, ROW], F16)
            nc.sync.dma_start(out=selfsb[:], in_=selfT.ap())
            validsb = res.tile([128, TPC], F32)
            nc.sync.dma_start(out=validsb[:], in_=validT.ap())

            dist2 = res.tile([128, TPC, JTP], F32)
            pm = res.tile([128, TPC, JTP], F32)
            nc.vector.memset(dist2[:], POIS_S)
            nc.vector.memset(pm[:], 0.0)

            selff32 = selfsb[:].bitcast(F32)        # (128, TPC, 64)

            qrr = 0
            off = 0
            for t in range(TPC):
                jt = int(JT[t])
                iv = ipool.tile([128, JTP * 8], I16, tag="i")
                nc.sync.dma_start(out=iv[:, 0 : jt * 8], in_=idxT.ap()[:, off : off + jt * 8])
                off += jt * 8
                g = gpool.tile([128, JTP, ROW], F16, tag="g")
                for w in range(NWIN):
                    jm = int(jmax[t, w])
                    if jm == 0:
                        continue
                    jo = int(joff[t, w])
                    nc.gpsimd.dma_gather(
                        out_ap=g[:, jo : jo + jm, :],
                        in_ap=tabT.ap()[w * WSTR : (w + 1) * WSTR, :],
                        idxs_ap=iv[:, jo * 8 : (jo + jm) * 8],
                        num_idxs=jm * 128,
                        num_idxs_reg=jm * 128,
                        elem_size=ROW,
                        single_packet=False,
                        queue_num=qrr,
                    )
                    qrr = (qrr + 1) % NQUEUES

                m = mpool.tile([128, JTP, C], F16, tag="m")
                fb = selfsb[:, t, 0:C].unsqueeze(1).broadcast_to([128, jt, C])
                nc.vector.tensor_tensor(
                    out=m[:, 0:jt, :], in0=g[:, 0:jt, 0:C], in1=fb, op=mybir.AluOpType.mult
                )
                nc.vector.tensor_add(
                    out=m[:, 0:jt, 0:32], in0=m[:, 0:jt, 0:32], in1=m[:, 0:jt, 32:64]
                )
                nc.vector.tensor_add(
                    out=m[:, 0:jt, 0:16], in0=m[:, 0:jt, 0:16], in1=m[:, 0:jt, 16:32]
                )
                nc.vector.tensor_add(
                    out=m[:, 0:jt, 0:8], in0=m[:, 0:jt, 0:8], in1=m[:, 0:jt, 8:16]
                )
                dslice = dist2[:, t, 0:jt]           # (128, jt)
                nc.vector.reduce_sum(
                    out=dslice, in_=m[:, 0:jt, 0:8], axis=mybir.AxisListType.X
                )
                gf32 = g[:].bitcast(F32)             # (128, JTP, 64)
                sj = gf32[:, 0:jt, 32]               # (128, jt)
                nc.vector.scalar_tensor_tensor(
                    out=dslice, in0=dslice, scalar=-2.0, in1=sj,
                    op0=mybir.AluOpType.mult, op1=mybir.AluOpType.add,
                )
                si = selff32[:, t, 32].unsqueeze(1).broadcast_to([128, jt])
                nc.vector.tensor_add(out=dslice, in0=dslice, in1=si)
                nl = g[:, 0:jt, 66]                  # (128, jt)
                li = selfsb[:, t, 66].unsqueeze(1).broadcast_to([128, jt])
                nc.vector.tensor_tensor(
                    out=pm[:, t, 0:jt], in0=nl, in1=li, op=mybir.AluOpType.is_equal,
                )

            # ---- phase 2 ----
            nc.scalar.sqrt(out=dist2[:], in_=dist2[:])
            mind = p2.tile([128, TPC], F32)
            nc.vector.tensor_reduce(
                out=mind[:], in_=dist2[:], axis=mybir.AxisListType.X,
                op=mybir.AluOpType.min,
            )
            mbc = mind[:].unsqueeze(2).broadcast_to([128, TPC, JTP])
            nc.vector.tensor_tensor(
                out=dist2[:], in0=dist2[:], in1=mbc, op=mybir.AluOpType.subtract
            )
            nc.scalar.activation(
                out=dist2[:], in_=dist2[:],
                func=mybir.ActivationFunctionType.Exp, scale=-1.0 / TEMP,
            )
            negs = p2.tile([128, TPC], F32)
            nc.vector.reduce_sum(out=negs[:], in_=dist2[:], axis=mybir.AxisListType.X)
            nc.vector.tensor_tensor(
                out=dist2[:], in0=dist2[:], in1=pm[:], op=mybir.AluOpType.mult
            )
            poss = p2.tile([128, TPC], F32)
            nc.vector.reduce_sum(out=poss[:], in_=dist2[:], axis=mybir.AxisListType.X)
            cnts = p2.tile([128, TPC], F32)
            nc.vector.reduce_sum(out=cnts[:], in_=pm[:], axis=mybir.AxisListType.X)
            rn = p2.tile([128, TPC], F32)
            nc.vector.reciprocal(out=rn[:], in_=negs[:])
            ratio = p2.tile([128, TPC], F32)
            nc.vector.tensor_tensor(
                out=ratio[:], in0=poss[:], in1=rn[:], op=mybir.AluOpType.mult
            )
            eps_t = p2.tile([128, 1], F32)
            nc.vector.memset(eps_t[:], EPS)
            lg = p2.tile([128, TPC], F32)
            nc.scalar.activation(
                out=lg[:], in_=ratio[:],
                func=mybir.ActivationFunctionType.Ln, bias=eps_t[:],
            )
            ma = p2.tile([128, TPC], F32)
            nc.vector.tensor_scalar(
                out=ma[:], in0=cnts[:], scalar1=0.5, scalar2=None,
                op0=mybir.AluOpType.is_gt,
            )
            mb2 = p2.tile([128, TPC], F32)
            nc.vector.tensor_scalar(
                out=mb2[:], in0=cnts[:], scalar1=float(K) - 0.5, scalar2=None,
                op0=mybir.AluOpType.is_lt,
            )
            nc.vector.tensor_tensor(
                out=ma[:], in0=ma[:], in1=mb2[:], op=mybir.AluOpType.mult
            )
            nc.vector.tensor_tensor(
                out=ma[:], in0=ma[:], in1=validsb[:], op=mybir.AluOpType.mult
            )
            nc.vector.tensor_tensor(
                out=lg[:], in0=lg[:], in1=ma[:], op=mybir.AluOpType.mult
            )
            outsb = p2.tile([128, 2], F32)
            nc.vector.reduce_sum(out=outsb[:, 0:1], in_=lg[:], axis=mybir.AxisListType.X)
            nc.vector.reduce_sum(out=outsb[:, 1:2], in_=ma[:], axis=mybir.AxisListType.X)
            nc.sync.dma_start(out=outT.ap(), in_=outsb[:])

    nc.compile()
    return nc


def run(features, labels, neighbor_idx, trace=False):
    packed, table = _pack_table(features, labels)
    plan = _build_plan(neighbor_idx)
    jmax, joff, JT, JTP = plan["jmax"], plan["joff"], plan["JT"], plan["JTP"]
    TOTW = int(JT.sum()) * 8

    key = (JTP, TOTW, jmax.tobytes())
    if _CACHE.get("key") != key:
        _CACHE["nc"] = _build(jmax, joff, JT, JTP, TOTW)
        _CACHE["key"] = key
    nc = _CACHE["nc"]

    in_maps = []
    for c in range(NCORES):
        m = _core_inputs(plan, packed, c)
        m["tab"] = table
        in_maps.append(m)
    res = bass_utils.run_bass_kernel_spmd(
        nc, in_maps, core_ids=list(range(NCORES)), trace=trace
    )
    s = 0.0
    ccnt = 0.0
    for o in res.results:
        s += float(o["out"][:, 0].astype(np.float64).sum())
        ccnt += float(o["out"][:, 1].astype(np.float64).sum())
    loss = np.float32(-s / max(ccnt, 1.0))
    return loss, res


def kernel(features, labels, neighbor_idx):
    loss, _ = run(features, labels, neighbor_idx, trace=False)
    return loss


# revision 4
# speedup vs baseline: 7.5343x; 1.2202x over previous
"""Trainium2 Bass kernel for ContrastHead (softnn contrastive KNN loss).

Data-parallel over points on 8 cores. Host packs a table row per point
(256B = 128 f16 slots): [f16 features(64) | f32 ||f||^2 | f16 label | pad].
Table = 4 windows of 32766 rows, each prefixed by a POISON row (features 0,
||f||^2 = 1e6, label -1) so int16 dma_gather indices address any row.
Points are globally sorted by their per-window neighbor-count profile and
dealt round-robin to cores, so all cores' tile-t lane profiles are nearly
identical and the shared per-tile per-window block sizes stay tight. Each
point's 31 neighbors are sorted by (window, index). Per tile, one gather per
window writes a dense j-slice of a single (128, JT_t, 256B) buffer — no
dummy-row traffic (~3.3x fewer descriptors than the fixed 4-window scheme).
Ragged slots gather the poison row, which self-masks: exp((min-1000)/T)=0
and label -1 never matches. Gathers round-robin the 4 SWDGE queues so
descriptor generation runs on all four Q7 core pairs concurrently; all
indices are preloaded to SBUF so gathers never wait on index DMAs.
Per tile: DVE f16 multiply + tree-add + reduce -> dots; dist^2 =
s_i + s_j - 2*dot; posmask via is_equal. Phase 2 on (128, TPC*JTP):
sqrt -> row min -> exp((min-d)/T) -> neg/pos sums -> ratio -> Ln(+1e-8) ->
point mask -> (128, 2) accumulators. Host sums 8x(128,2).
"""

import numpy as np

import concourse.bacc as bacc
import concourse.bass as bass
import concourse.mybir as mybir
import concourse.tile as tile
from concourse import bass_utils

F16 = mybir.dt.float16
F32 = mybir.dt.float32
I16 = mybir.dt.int16

N = 100000
K = 31
C = 64
ROW = 128                   # f16 slots per table row (256B)
WINR = 32766                # real rows per window
WSTR = WINR + 1             # window stride in table (incl poison row 0)
NWIN = 4
TABROWS = NWIN * WSTR
NCORES = 8
PTS = N // NCORES           # 12500
TPC = (PTS + 127) // 128    # 98 tiles/core
TEMP = 0.1
EPS = 1e-8
POIS_S = 1.0e6
NQUEUES = 4

_CACHE = {}


# ---------------- host-side plan + packing ----------------

def _pack_table(features, labels):
    packed = np.zeros((N, ROW), dtype=np.float16)
    packed[:, 0:C] = features.astype(np.float16)
    s = np.sum(features.astype(np.float64) ** 2, axis=1).astype(np.float32)
    packed[:, 64:66] = s[:, None].view(np.float16)
    packed[:, 66] = labels.astype(np.float16)

    tab = np.zeros((TABROWS, ROW), dtype=np.float16)
    pr = np.zeros((ROW,), dtype=np.float16)
    pr[64:66] = np.array([POIS_S], dtype=np.float32).view(np.float16)
    pr[66] = -1.0
    for w in range(NWIN):
        lo = w * WINR
        hi = min(lo + WINR, N)
        tab[w * WSTR] = pr
        if hi > lo:
            tab[w * WSTR + 1 : w * WSTR + 1 + (hi - lo)] = packed[lo:hi]
    return packed, tab


def _build_plan(neighbor_idx):
    """Global profile-sorted round-robin sharding + shared gather plan."""
    w_all = (neighbor_idx // WINR).astype(np.int32)
    loc_all = (neighbor_idx - w_all * WINR + 1).astype(np.int32)

    cnt = np.stack([(w_all == j).sum(1) for j in range(NWIN)], axis=1).astype(np.int32)
    order = np.lexsort((cnt[:, 3], cnt[:, 1], cnt[:, 0]))          # (N,) global
    # neighbors of each point sorted by (window, index) for HBM locality
    ks = np.argsort(w_all * 32768 + loc_all, axis=1)
    loc_s = np.take_along_axis(loc_all, ks, axis=1)
    cum = np.zeros((N, NWIN), np.int32)
    cum[:, 1:] = np.cumsum(cnt, axis=1)[:, :-1]

    # shared per-tile blocks: tile t holds sorted ranks [t*1024, (t+1)*1024)
    padN = TPC * 128 * NCORES
    cnt_p = np.zeros((padN, NWIN), np.int32)
    cnt_p[:N] = cnt[order]
    jmax = cnt_p.reshape(TPC, 128 * NCORES, NWIN).max(axis=1)       # (TPC, NWIN)
    joff = np.zeros((TPC, NWIN), np.int32)
    joff[:, 1:] = np.cumsum(jmax, axis=1)[:, :-1]
    JT = jmax.sum(axis=1)
    return dict(jmax=jmax, joff=joff, JT=JT, JTP=int(JT.max()),
                order=order, cnt=cnt, cum=cum, loc_s=loc_s)


def _core_inputs(plan, packed, c):
    order, cnt, cum, loc_s = plan["order"], plan["cnt"], plan["cum"], plan["loc_s"]
    jmax, JT = plan["jmax"], plan["JT"]
    # core c, slot q (= t*128+p) <- global sorted rank q*8+c
    ranks = np.arange(TPC * 128) * NCORES + c
    real = ranks < N
    pts = np.where(real, order[np.minimum(ranks, N - 1)], 0)        # (TPC*128,)

    segs = []
    for t in range(TPC):
        pts_t = pts[t * 128 : (t + 1) * 128]
        real_t = real[t * 128 : (t + 1) * 128]
        tile_flat = []
        for w in range(NWIN):
            jm = int(jmax[t, w])
            if jm == 0:
                continue
            n_w = np.where(real_t, cnt[pts_t, w], 0)
            st = cum[pts_t, w]
            col = st[:, None] + np.arange(jm)[None, :]
            validm = np.arange(jm)[None, :] < n_w[:, None]
            vals = np.where(
                validm,
                np.take_along_axis(loc_s[pts_t], np.minimum(col, K - 1), axis=1),
                0,
            )
            tile_flat.append(vals.T.reshape(-1))     # j-major flat
        flat = np.concatenate(tile_flat).astype(np.int16)
        wrapped = flat.reshape(-1, 16).T             # (16, JT_t*8)
        segs.append(np.tile(wrapped, (8, 1)))        # (128, JT_t*8)
    idx16 = np.ascontiguousarray(np.concatenate(segs, axis=1))

    rows = np.where(real[:, None], packed[pts], 0).astype(np.float16)
    selftab = np.ascontiguousarray(rows.reshape(TPC, 128, ROW).transpose(1, 0, 2))
    valid = real.astype(np.float32)
    valid = np.ascontiguousarray(valid.reshape(TPC, 128).transpose(1, 0))
    return dict(selftab=selftab, nidx16=idx16, valid=valid)


# ---------------- device program ----------------

def _build(jmax, joff, JT, JTP, TOTW):
    nc = bacc.Bacc(
        "TRN2", target_bir_lowering=False, debug=False, num_swdge_queues=NQUEUES
    )

    tabT = nc.dram_tensor("tab", (TABROWS, ROW), F16, kind="ExternalInput")
    selfT = nc.dram_tensor("selftab", (128, TPC, ROW), F16, kind="ExternalInput")
    idxT = nc.dram_tensor("nidx16", (128, TOTW), I16, kind="ExternalInput")
    validT = nc.dram_tensor("valid", (128, TPC), F32, kind="ExternalInput")
    outT = nc.dram_tensor("out", (128, 2), F32, kind="ExternalOutput")

    with tile.TileContext(nc) as tc:
        with (
            tc.tile_pool(name="res", bufs=1) as res,
            tc.tile_pool(name="gpool", bufs=3) as gpool,
            tc.tile_pool(name="mpool", bufs=3) as mpool,
            tc.tile_pool(name="p2", bufs=1) as p2,
        ):
            selfsb = res.tile([128, TPC, ROW], F16)
            nc.sync.dma_start(out=selfsb[:], in_=selfT.ap())
            validsb = res.tile([128, TPC], F32)
            nc.sync.dma_start(out=validsb[:], in_=validT.ap())
            idxsb = res.tile([128, TOTW], I16)
            nc.sync.dma_start(out=idxsb[:], in_=idxT.ap())

            dist2 = res.tile([128, TPC, JTP], F32)
            pm = res.tile([128, TPC, JTP], F32)
            nc.vector.memset(dist2[:], POIS_S)
            nc.vector.memset(pm[:], 0.0)

            selff32 = selfsb[:].bitcast(F32)        # (128, TPC, 64)

            qrr = 0
            off = 0
            for t in range(TPC):
                jt = int(JT[t])
                g = gpool.tile([128, JTP, ROW], F16, tag="g")
                for w in range(NWIN):
                    jm = int(jmax[t, w])
                    if jm == 0:
                        continue
                    jo = int(joff[t, w])
                    nc.gpsimd.dma_gather(
                        out_ap=g[:, jo : jo + jm, :],
                        in_ap=tabT.ap()[w * WSTR : (w + 1) * WSTR, :],
                        idxs_ap=idxsb[:, off + jo * 8 : off + (jo + jm) * 8],
                        num_idxs=jm * 128,
                        num_idxs_reg=jm * 128,
                        elem_size=ROW,
                        single_packet=False,
                        queue_num=qrr,
                    )
                    qrr = (qrr + 1) % NQUEUES
                off += jt * 8

                m = mpool.tile([128, JTP, C], F16, tag="m")
                fb = selfsb[:, t, 0:C].unsqueeze(1).broadcast_to([128, jt, C])
                nc.vector.tensor_tensor(
                    out=m[:, 0:jt, :], in0=g[:, 0:jt, 0:C], in1=fb, op=mybir.AluOpType.mult
                )
                nc.vector.tensor_add(
                    out=m[:, 0:jt, 0:32], in0=m[:, 0:jt, 0:32], in1=m[:, 0:jt, 32:64]
                )
                nc.vector.tensor_add(
                    out=m[:, 0:jt, 0:16], in0=m[:, 0:jt, 0:16], in1=m[:, 0:jt, 16:32]
                )
                nc.vector.tensor_add(
                    out=m[:, 0:jt, 0:8], in0=m[:, 0:jt, 0:8], in1=m[:, 0:jt, 8:16]
                )
                dslice = dist2[:, t, 0:jt]           # (128, jt)
                nc.vector.reduce_sum(
                    out=dslice, in_=m[:, 0:jt, 0:8], axis=mybir.AxisListType.X
                )
                gf32 = g[:].bitcast(F32)             # (128, JTP, 64)
                sj = gf32[:, 0:jt, 32]               # (128, jt)
                nc.vector.scalar_tensor_tensor(
                    out=dslice, in0=dslice, scalar=-2.0, in1=sj,
                    op0=mybir.AluOpType.mult, op1=mybir.AluOpType.add,
                )
                si = selff32[:, t, 32].unsqueeze(1).broadcast_to([128, jt])
                nc.vector.tensor_add(out=dslice, in0=dslice, in1=si)
                nl = g[:, 0:jt, 66]                  # (128, jt)
                li = selfsb[:, t, 66].unsqueeze(1).broadcast_to([128, jt])
                nc.vector.tensor_tensor(
                    out=pm[:, t, 0:jt], in0=nl, in1=li, op=mybir.AluOpType.is_equal,
                )

            # ---- phase 2 ----
            nc.scalar.sqrt(out=dist2[:], in_=dist2[:])
            mind = p2.tile([128, TPC], F32)
            nc.vector.tensor_reduce(
                out=mind[:], in_=dist2[:], axis=mybir.AxisListType.X,
                op=mybir.AluOpType.min,
            )
            mbc = mind[:].unsqueeze(2).broadcast_to([128, TPC, JTP])
            nc.vector.tensor_tensor(
                out=dist2[:], in0=dist2[:], in1=mbc, op=mybir.AluOpType.subtract
            )
            nc.scalar.activation(
                out=dist2[:], in_=dist2[:],
                func=mybir.ActivationFunctionType.Exp, scale=-1.0 / TEMP,
            )
            negs = p2.tile([128, TPC], F32)
            nc.vector.reduce_sum(out=negs[:], in_=dist2[:], axis=mybir.AxisListType.X)
            nc.vector.tensor_tensor(
                out=dist2[:], in0=dist2[:], in1=pm[:], op=mybir.AluOpType.mult
            )
            poss = p2.tile([128, TPC], F32)
            nc.vector.reduce_sum(out=poss[:], in_=dist2[:], axis=mybir.AxisListType.X)
            cnts = p2.tile([128, TPC], F32)
            nc.vector.reduce_sum(out=cnts[:], in_=pm[:], axis=mybir.AxisListType.X)
            rn = p2.tile([128, TPC], F32)
            nc.vector.reciprocal(out=rn[:], in_=negs[:])
            ratio = p2.tile([128, TPC], F32)
            nc.vector.tensor_tensor(
                out=ratio[:], in0=poss[:], in1=rn[:], op=mybir.AluOpType.mult
            )
            eps_t = p2.tile([128, 1], F32)
            nc.vector.memset(eps_t[:], EPS)
            lg = p2.tile([128, TPC], F32)
            nc.scalar.activation(
                out=lg[:], in_=ratio[:],
                func=mybir.ActivationFunctionType.Ln, bias=eps_t[:],
            )
            ma = p2.tile([128, TPC], F32)
            nc.vector.tensor_scalar(
                out=ma[:], in0=cnts[:], scalar1=0.5, scalar2=None,
                op0=mybir.AluOpType.is_gt,
            )
            mb2 = p2.tile([128, TPC], F32)
            nc.vector.tensor_scalar(
                out=mb2[:], in0=cnts[:], scalar1=float(K) - 0.5, scalar2=None,
                op0=mybir.AluOpType.is_lt,
            )
            nc.vector.tensor_tensor(
                out=ma[:], in0=ma[:], in1=mb2[:], op=mybir.AluOpType.mult
            )
            nc.vector.tensor_tensor(
                out=ma[:], in0=ma[:], in1=validsb[:], op=mybir.AluOpType.mult
            )
            nc.vector.tensor_tensor(
                out=lg[:], in0=lg[:], in1=ma[:], op=mybir.AluOpType.mult
            )
            outsb = p2.tile([128, 2], F32)
            nc.vector.reduce_sum(out=outsb[:, 0:1], in_=lg[:], axis=mybir.AxisListType.X)
            nc.vector.reduce_sum(out=outsb[:, 1:2], in_=ma[:], axis=mybir.AxisListType.X)
            nc.sync.dma_start(out=outT.ap(), in_=outsb[:])

    nc.compile()
    return nc


def run(features, labels, neighbor_idx, trace=False):
    packed, table = _pack_table(features, labels)
    plan = _build_plan(neighbor_idx)
    jmax, joff, JT, JTP = plan["jmax"], plan["joff"], plan["JT"], plan["JTP"]
    TOTW = int(JT.sum()) * 8

    key = (JTP, TOTW, jmax.tobytes())
    if _CACHE.get("key") != key:
        _CACHE["nc"] = _build(jmax, joff, JT, JTP, TOTW)
        _CACHE["key"] = key
    nc = _CACHE["nc"]

    in_maps = []
    for c in range(NCORES):
        m = _core_inputs(plan, packed, c)
        m["tab"] = table
        in_maps.append(m)
    res = bass_utils.run_bass_kernel_spmd(
        nc, in_maps, core_ids=list(range(NCORES)), trace=trace
    )
    s = 0.0
    ccnt = 0.0
    for o in res.results:
        s += float(o["out"][:, 0].astype(np.float64).sum())
        ccnt += float(o["out"][:, 1].astype(np.float64).sum())
    loss = np.float32(-s / max(ccnt, 1.0))
    return loss, res


def kernel(features, labels, neighbor_idx):
    loss, _ = run(features, labels, neighbor_idx, trace=False)
    return loss


# revision 7
# speedup vs baseline: 7.8194x; 1.0378x over previous
"""Trainium2 Bass kernel for ContrastHead (softnn contrastive KNN loss).

Data-parallel over points on 8 cores. Host packs a table row per point
(256B = 128 f16 slots): [f16 features(64) | f32 ||f||^2 | f16 label | pad].
Table = 4 windows of 32766 rows, each prefixed by a POISON row (features 0,
||f||^2 = 1e6, label -1) so int16 dma_gather indices address any row.
Points are globally sorted by their per-window neighbor-count profile and
dealt round-robin to cores, so all cores' tile-t lane profiles are nearly
identical and the shared per-tile per-window block sizes stay tight. Each
point's 31 neighbors are sorted by (window, index). Per tile, one gather per
window writes a dense j-slice of a single (128, JT_t, 256B) buffer — no
dummy-row traffic (~3.3x fewer descriptors than the fixed 4-window scheme).
Ragged slots gather the poison row, which self-masks: exp((min-1000)/T)=0
and label -1 never matches. Gathers round-robin the 4 SWDGE queues so
descriptor generation runs on all four Q7 core pairs concurrently; all
indices are preloaded to SBUF so gathers never wait on index DMAs.
Per tile: DVE f16 multiply + tree-add + reduce -> dots; dist^2 =
s_i + s_j - 2*dot; posmask via is_equal. Phase 2 on (128, TPC*JTP):
sqrt -> row min -> exp((min-d)/T) -> neg/pos sums -> ratio -> Ln(+1e-8) ->
point mask -> (128, 2) accumulators. Host sums 8x(128,2).
"""

import numpy as np

import concourse.bacc as bacc
import concourse.bass as bass
import concourse.mybir as mybir
import concourse.tile as tile
from concourse import bass_utils

F16 = mybir.dt.float16
F32 = mybir.dt.float32
I16 = mybir.dt.int16

N = 100000
K = 31
C = 64
ROW = 128                   # f16 slots per table row (256B)
WINR = 32766                # real rows per window
WSTR = WINR + 1             # window stride in table (incl poison row 0)
NWIN = 4
TABROWS = NWIN * WSTR
NCORES = 8
PTS = N // NCORES           # 12500
TPC = (PTS + 127) // 128    # 98 tiles/core
TEMP = 0.1
EPS = 1e-8
POIS_S = 1.0e6
NQUEUES = 4

_CACHE = {}


# ---------------- host-side plan + packing ----------------

def _pack_table(features, labels):
    packed = np.zeros((N, ROW), dtype=np.float16)
    packed[:, 0:C] = features.astype(np.float16)
    s = np.sum(features.astype(np.float64) ** 2, axis=1).astype(np.float32)
    packed[:, 64:66] = s[:, None].view(np.float16)
    packed[:, 66] = labels.astype(np.float16)

    tab = np.zeros((TABROWS, ROW), dtype=np.float16)
    pr = np.zeros((ROW,), dtype=np.float16)
    pr[64:66] = np.array([POIS_S], dtype=np.float32).view(np.float16)
    pr[66] = -1.0
    for w in range(NWIN):
        lo = w * WINR
        hi = min(lo + WINR, N)
        tab[w * WSTR] = pr
        if hi > lo:
            tab[w * WSTR + 1 : w * WSTR + 1 + (hi - lo)] = packed[lo:hi]
    return packed, tab


def _build_plan(neighbor_idx):
    """Global profile-sorted round-robin sharding + shared gather plan."""
    w_all = (neighbor_idx // WINR).astype(np.int32)
    loc_all = (neighbor_idx - w_all * WINR + 1).astype(np.int32)

    cnt = np.stack([(w_all == j).sum(1) for j in range(NWIN)], axis=1).astype(np.int32)
    order = np.lexsort((cnt[:, 3], cnt[:, 1], cnt[:, 0]))          # (N,) global
    # neighbors of each point sorted by (window, index) for HBM locality
    ks = np.argsort(w_all * 32768 + loc_all, axis=1)
    loc_s = np.take_along_axis(loc_all, ks, axis=1)
    cum = np.zeros((N, NWIN), np.int32)
    cum[:, 1:] = np.cumsum(cnt, axis=1)[:, :-1]

    # shared per-tile blocks: tile t holds sorted ranks [t*1024, (t+1)*1024)
    padN = TPC * 128 * NCORES
    cnt_p = np.zeros((padN, NWIN), np.int32)
    cnt_p[:N] = cnt[order]
    jmax = cnt_p.reshape(TPC, 128 * NCORES, NWIN).max(axis=1)       # (TPC, NWIN)
    joff = np.zeros((TPC, NWIN), np.int32)
    joff[:, 1:] = np.cumsum(jmax, axis=1)[:, :-1]
    JT = jmax.sum(axis=1)
    return dict(jmax=jmax, joff=joff, JT=JT, JTP=int(JT.max()),
                order=order, cnt=cnt, cum=cum, loc_s=loc_s)


def _core_inputs(plan, packed, c):
    order, cnt, cum, loc_s = plan["order"], plan["cnt"], plan["cum"], plan["loc_s"]
    jmax, JT = plan["jmax"], plan["JT"]
    # core c, slot q (= t*128+p) <- global sorted rank q*8+c
    ranks = np.arange(TPC * 128) * NCORES + c
    real = ranks < N
    pts = np.where(real, order[np.minimum(ranks, N - 1)], 0)        # (TPC*128,)

    segs = []
    for t in range(TPC):
        pts_t = pts[t * 128 : (t + 1) * 128]
        real_t = real[t * 128 : (t + 1) * 128]
        tile_flat = []
        for w in range(NWIN):
            jm = int(jmax[t, w])
            if jm == 0:
                continue
            n_w = np.where(real_t, cnt[pts_t, w], 0)
            st = cum[pts_t, w]
            col = st[:, None] + np.arange(jm)[None, :]
            validm = np.arange(jm)[None, :] < n_w[:, None]
            vals = np.where(
                validm,
                np.take_along_axis(loc_s[pts_t], np.minimum(col, K - 1), axis=1),
                0,
            )
            tile_flat.append(vals.T.reshape(-1))     # j-major flat
        flat = np.concatenate(tile_flat).astype(np.int16)
        wrapped = flat.reshape(-1, 16).T             # (16, JT_t*8)
        segs.append(np.tile(wrapped, (8, 1)))        # (128, JT_t*8)
    idx16 = np.ascontiguousarray(np.concatenate(segs, axis=1))

    rows = np.where(real[:, None], packed[pts], 0).astype(np.float16)
    selftab = np.ascontiguousarray(rows.reshape(TPC, 128, ROW).transpose(1, 0, 2))
    valid = real.astype(np.float32)
    valid = np.ascontiguousarray(valid.reshape(TPC, 128).transpose(1, 0))
    return dict(selftab=selftab, nidx16=idx16, valid=valid)


# ---------------- device program ----------------

def _build(jmax, joff, JT, JTP, TOTW):
    nc = bacc.Bacc(
        "TRN2", target_bir_lowering=False, debug=False, num_swdge_queues=NQUEUES
    )

    tabT = nc.dram_tensor("tab", (TABROWS, ROW), F16, kind="ExternalInput")
    selfT = nc.dram_tensor("selftab", (128, TPC, ROW), F16, kind="ExternalInput")
    idxT = nc.dram_tensor("nidx16", (128, TOTW), I16, kind="ExternalInput")
    validT = nc.dram_tensor("valid", (128, TPC), F32, kind="ExternalInput")
    outT = nc.dram_tensor("out", (128, 2), F32, kind="ExternalOutput")

    with tile.TileContext(nc) as tc:
        with (
            tc.tile_pool(name="res", bufs=1) as res,
            tc.tile_pool(name="gpool", bufs=3) as gpool,
            tc.tile_pool(name="mpool", bufs=3) as mpool,
            tc.tile_pool(name="p2", bufs=1) as p2,
        ):
            selfsb = res.tile([128, TPC, ROW], F16)
            nc.sync.dma_start(out=selfsb[:], in_=selfT.ap())
            validsb = res.tile([128, TPC], F32)
            nc.sync.dma_start(out=validsb[:], in_=validT.ap())
            idxsb = res.tile([128, TOTW], I16)
            nc.sync.dma_start(out=idxsb[:], in_=idxT.ap())

            dist2 = res.tile([128, TPC, JTP], F32)
            pm = res.tile([128, TPC, JTP], F32)
            nc.vector.memset(dist2[:], POIS_S)
            nc.vector.memset(pm[:], 0.0)

            selff32 = selfsb[:].bitcast(F32)        # (128, TPC, 64)

            qrr = 0
            off = 0
            for t in range(TPC):
                jt = int(JT[t])
                g = gpool.tile([128, JTP, ROW], F16, tag="g")
                for w in range(NWIN):
                    jm = int(jmax[t, w])
                    if jm == 0:
                        continue
                    jo = int(joff[t, w])
                    nc.gpsimd.dma_gather(
                        out_ap=g[:, jo : jo + jm, :],
                        in_ap=tabT.ap()[w * WSTR : (w + 1) * WSTR, :],
                        idxs_ap=idxsb[:, off + jo * 8 : off + (jo + jm) * 8],
                        num_idxs=jm * 128,
                        num_idxs_reg=jm * 128,
                        elem_size=ROW,
                        single_packet=False,
                        queue_num=qrr,
                    )
                    qrr = (qrr + 1) % NQUEUES
                off += jt * 8

                m = mpool.tile([128, JTP, C], F16, tag="m")
                fb = selfsb[:, t, 0:C].unsqueeze(1).broadcast_to([128, jt, C])
                nc.vector.tensor_tensor(
                    out=m[:, 0:jt, :], in0=g[:, 0:jt, 0:C], in1=fb, op=mybir.AluOpType.mult
                )
                dslice = dist2[:, t, 0:jt]           # (128, jt)
                nc.vector.reduce_sum(
                    out=dslice, in_=m[:, 0:jt, :], axis=mybir.AxisListType.X
                )
                gf32 = g[:].bitcast(F32)             # (128, JTP, 64)
                sj = gf32[:, 0:jt, 32]               # (128, jt)
                nc.vector.scalar_tensor_tensor(
                    out=dslice, in0=dslice, scalar=-2.0, in1=sj,
                    op0=mybir.AluOpType.mult, op1=mybir.AluOpType.add,
                )
                si = selff32[:, t, 32].unsqueeze(1).broadcast_to([128, jt])
                nc.vector.tensor_add(out=dslice, in0=dslice, in1=si)
                nl = g[:, 0:jt, 66]                  # (128, jt)
                li = selfsb[:, t, 66].unsqueeze(1).broadcast_to([128, jt])
                nc.vector.tensor_tensor(
                    out=pm[:, t, 0:jt], in0=nl, in1=li, op=mybir.AluOpType.is_equal,
                )

            # ---- phase 2 ----
            nc.scalar.sqrt(out=dist2[:], in_=dist2[:])
            mind = p2.tile([128, TPC], F32)
            nc.vector.tensor_reduce(
                out=mind[:], in_=dist2[:], axis=mybir.AxisListType.X,
                op=mybir.AluOpType.min,
            )
            mbc = mind[:].unsqueeze(2).broadcast_to([128, TPC, JTP])
            nc.vector.tensor_tensor(
                out=dist2[:], in0=dist2[:], in1=mbc, op=mybir.AluOpType.subtract
            )
            nc.scalar.activation(
                out=dist2[:], in_=dist2[:],
                func=mybir.ActivationFunctionType.Exp, scale=-1.0 / TEMP,
            )
            negs = p2.tile([128, TPC], F32)
            nc.vector.reduce_sum(out=negs[:], in_=dist2[:], axis=mybir.AxisListType.X)
            nc.vector.tensor_tensor(
                out=dist2[:], in0=dist2[:], in1=pm[:], op=mybir.AluOpType.mult
            )
            poss = p2.tile([128, TPC], F32)
            nc.vector.reduce_sum(out=poss[:], in_=dist2[:], axis=mybir.AxisListType.X)
            cnts = p2.tile([128, TPC], F32)
            nc.vector.reduce_sum(out=cnts[:], in_=pm[:], axis=mybir.AxisListType.X)
            rn = p2.tile([128, TPC], F32)
            nc.vector.reciprocal(out=rn[:], in_=negs[:])
            ratio = p2.tile([128, TPC], F32)
            nc.vector.tensor_tensor(
                out=ratio[:], in0=poss[:], in1=rn[:], op=mybir.AluOpType.mult
            )
            eps_t = p2.tile([128, 1], F32)
            nc.vector.memset(eps_t[:], EPS)
            lg = p2.tile([128, TPC], F32)
            nc.scalar.activation(
                out=lg[:], in_=ratio[:],
                func=mybir.ActivationFunctionType.Ln, bias=eps_t[:],
            )
            ma = p2.tile([128, TPC], F32)
            nc.vector.tensor_scalar(
                out=ma[:], in0=cnts[:], scalar1=0.5, scalar2=None,
                op0=mybir.AluOpType.is_gt,
            )
            mb2 = p2.tile([128, TPC], F32)
            nc.vector.tensor_scalar(
                out=mb2[:], in0=cnts[:], scalar1=float(K) - 0.5, scalar2=None,
                op0=mybir.AluOpType.is_lt,
            )
            nc.vector.tensor_tensor(
                out=ma[:], in0=ma[:], in1=mb2[:], op=mybir.AluOpType.mult
            )
            nc.vector.tensor_tensor(
                out=ma[:], in0=ma[:], in1=validsb[:], op=mybir.AluOpType.mult
            )
            nc.vector.tensor_tensor(
                out=lg[:], in0=lg[:], in1=ma[:], op=mybir.AluOpType.mult
            )
            outsb = p2.tile([128, 2], F32)
            nc.vector.reduce_sum(out=outsb[:, 0:1], in_=lg[:], axis=mybir.AxisListType.X)
            nc.vector.reduce_sum(out=outsb[:, 1:2], in_=ma[:], axis=mybir.AxisListType.X)
            nc.sync.dma_start(out=outT.ap(), in_=outsb[:])

    nc.compile()
    return nc


def run(features, labels, neighbor_idx, trace=False):
    packed, table = _pack_table(features, labels)
    plan = _build_plan(neighbor_idx)
    jmax, joff, JT, JTP = plan["jmax"], plan["joff"], plan["JT"], plan["JTP"]
    TOTW = int(JT.sum()) * 8

    key = (JTP, TOTW, jmax.tobytes())
    if _CACHE.get("key") != key:
        _CACHE["nc"] = _build(jmax, joff, JT, JTP, TOTW)
        _CACHE["key"] = key
    nc = _CACHE["nc"]

    in_maps = []
    for c in range(NCORES):
        m = _core_inputs(plan, packed, c)
        m["tab"] = table
        in_maps.append(m)
    res = bass_utils.run_bass_kernel_spmd(
        nc, in_maps, core_ids=list(range(NCORES)), trace=trace
    )
    s = 0.0
    ccnt = 0.0
    for o in res.results:
        s += float(o["out"][:, 0].astype(np.float64).sum())
        ccnt += float(o["out"][:, 1].astype(np.float64).sum())
    loss = np.float32(-s / max(ccnt, 1.0))
    return loss, res


def kernel(features, labels, neighbor_idx):
    loss, _ = run(features, labels, neighbor_idx, trace=False)
    return loss


# revision 8
# speedup vs baseline: 8.6763x; 1.1096x over previous
"""Trainium2 Bass kernel for ContrastHead (softnn contrastive KNN loss).

Data-parallel over points on 8 cores. Host packs a table row per point
(256B = 128 f16 slots): [f16 features(64) | f32 ||f||^2 | f16 label | pad].
Table = 4 windows of 32766 rows, each prefixed by a POISON row (features 0,
||f||^2 = 1e6, label -1) so int16 dma_gather indices address any row.
Points are globally sorted by their per-window neighbor-count profile and
dealt round-robin to cores, so all cores' tile-t lane profiles are nearly
identical and the shared per-tile per-window block sizes stay tight. Each
point's 31 neighbors are sorted by (window, index). Per tile, one gather per
window writes a dense j-slice of a single (128, JT_t, 256B) buffer — no
dummy-row traffic (~3.3x fewer descriptors than the fixed 4-window scheme).
Ragged slots gather the poison row, which self-masks: exp((min-1000)/T)=0
and label -1 never matches. Gathers round-robin the 4 SWDGE queues so
descriptor generation runs on all four Q7 core pairs concurrently; all
indices are preloaded to SBUF so gathers never wait on index DMAs.
Per tile: DVE f16 multiply + tree-add + reduce -> dots; dist^2 =
s_i + s_j - 2*dot; posmask via is_equal. Phase 2 on (128, TPC*JTP):
sqrt -> row min -> exp((min-d)/T) -> neg/pos sums -> ratio -> Ln(+1e-8) ->
point mask -> (128, 2) accumulators. Host sums 8x(128,2).
"""

import numpy as np

import concourse.bacc as bacc
import concourse.bass as bass
import concourse.mybir as mybir
import concourse.tile as tile
from concourse import bass_utils

F16 = mybir.dt.float16
F32 = mybir.dt.float32
I16 = mybir.dt.int16

N = 100000
K = 31
C = 64
ROW = 128                   # f16 slots per table row (256B)
WINR = 32766                # real rows per window
WSTR = WINR + 1             # window stride in table (incl poison row 0)
NWIN = 4
TABROWS = NWIN * WSTR
NCORES = 8
PTS = N // NCORES           # 12500
TPC = (PTS + 127) // 128    # 98 tiles/core
TEMP = 0.1
EPS = 1e-8
POIS_S = 1.0e6
NQUEUES = 4

_CACHE = {}


# ---------------- host-side plan + packing ----------------

def _pack_table(features, labels):
    packed = np.zeros((N, ROW), dtype=np.float16)
    packed[:, 0:C] = features.astype(np.float16)
    s = np.sum(features.astype(np.float64) ** 2, axis=1).astype(np.float32)
    packed[:, 64:66] = s[:, None].view(np.float16)
    packed[:, 66] = labels.astype(np.float16)

    tab = np.zeros((TABROWS, ROW), dtype=np.float16)
    pr = np.zeros((ROW,), dtype=np.float16)
    pr[64:66] = np.array([POIS_S], dtype=np.float32).view(np.float16)
    pr[66] = -1.0
    for w in range(NWIN):
        lo = w * WINR
        hi = min(lo + WINR, N)
        tab[w * WSTR] = pr
        if hi > lo:
            tab[w * WSTR + 1 : w * WSTR + 1 + (hi - lo)] = packed[lo:hi]
    return packed, tab


def _build_plan(neighbor_idx):
    """Global profile-sorted round-robin sharding + shared gather plan."""
    w_all = (neighbor_idx // WINR).astype(np.int32)
    loc_all = (neighbor_idx - w_all * WINR + 1).astype(np.int32)

    cnt = np.stack([(w_all == j).sum(1) for j in range(NWIN)], axis=1).astype(np.int32)
    order = np.lexsort((cnt[:, 3], cnt[:, 1], cnt[:, 0]))          # (N,) global
    # neighbors of each point sorted by (window, index) for HBM locality
    ks = np.argsort(w_all * 32768 + loc_all, axis=1)
    loc_s = np.take_along_axis(loc_all, ks, axis=1)
    cum = np.zeros((N, NWIN), np.int32)
    cum[:, 1:] = np.cumsum(cnt, axis=1)[:, :-1]

    # shared per-tile blocks: tile t holds sorted ranks [t*1024, (t+1)*1024)
    padN = TPC * 128 * NCORES
    cnt_p = np.zeros((padN, NWIN), np.int32)
    cnt_p[:N] = cnt[order]
    jmax = cnt_p.reshape(TPC, 128 * NCORES, NWIN).max(axis=1)       # (TPC, NWIN)
    joff = np.zeros((TPC, NWIN), np.int32)
    joff[:, 1:] = np.cumsum(jmax, axis=1)[:, :-1]
    JT = jmax.sum(axis=1)
    return dict(jmax=jmax, joff=joff, JT=JT, JTP=int(JT.max()),
                order=order, cnt=cnt, cum=cum, loc_s=loc_s)


def _core_inputs(plan, packed, c):
    order, cnt, cum, loc_s = plan["order"], plan["cnt"], plan["cum"], plan["loc_s"]
    jmax, JT = plan["jmax"], plan["JT"]
    # core c, slot q (= t*128+p) <- global sorted rank q*8+c
    ranks = np.arange(TPC * 128) * NCORES + c
    real = ranks < N
    pts = np.where(real, order[np.minimum(ranks, N - 1)], 0)        # (TPC*128,)

    segs = []
    for t in range(TPC):
        pts_t = pts[t * 128 : (t + 1) * 128]
        real_t = real[t * 128 : (t + 1) * 128]
        tile_flat = []
        for w in range(NWIN):
            jm = int(jmax[t, w])
            if jm == 0:
                continue
            n_w = np.where(real_t, cnt[pts_t, w], 0)
            st = cum[pts_t, w]
            col = st[:, None] + np.arange(jm)[None, :]
            validm = np.arange(jm)[None, :] < n_w[:, None]
            vals = np.where(
                validm,
                np.take_along_axis(loc_s[pts_t], np.minimum(col, K - 1), axis=1),
                0,
            )
            tile_flat.append(vals.T.reshape(-1))     # j-major flat
        flat = np.concatenate(tile_flat).astype(np.int16)
        wrapped = flat.reshape(-1, 16).T             # (16, JT_t*8)
        segs.append(np.tile(wrapped, (8, 1)))        # (128, JT_t*8)
    idx16 = np.ascontiguousarray(np.concatenate(segs, axis=1))

    rows = np.where(real[:, None], packed[pts], 0).astype(np.float16)
    selftab = np.ascontiguousarray(rows.reshape(TPC, 128, ROW).transpose(1, 0, 2))
    valid = real.astype(np.float32)
    valid = np.ascontiguousarray(valid.reshape(TPC, 128).transpose(1, 0))
    return dict(selftab=selftab, nidx16=idx16, valid=valid)


# ---------------- device program ----------------

def _build(jmax, joff, JT, JTP, TOTW):
    nc = bacc.Bacc(
        "TRN2", target_bir_lowering=False, debug=False, num_swdge_queues=NQUEUES
    )

    tabT = nc.dram_tensor("tab", (TABROWS, ROW), F16, kind="ExternalInput")
    selfT = nc.dram_tensor("selftab", (128, TPC, ROW), F16, kind="ExternalInput")
    idxT = nc.dram_tensor("nidx16", (128, TOTW), I16, kind="ExternalInput")
    validT = nc.dram_tensor("valid", (128, TPC), F32, kind="ExternalInput")
    outT = nc.dram_tensor("out", (128, 2), F32, kind="ExternalOutput")

    with tile.TileContext(nc) as tc:
        with (
            tc.tile_pool(name="res", bufs=1) as res,
            tc.tile_pool(name="gpool", bufs=4) as gpool,
            tc.tile_pool(name="mpool", bufs=2) as mpool,
            tc.tile_pool(name="p2", bufs=1) as p2,
        ):
            selfsb = res.tile([128, TPC, ROW], F16)
            nc.sync.dma_start(out=selfsb[:], in_=selfT.ap())
            validsb = res.tile([128, TPC], F32)
            nc.sync.dma_start(out=validsb[:], in_=validT.ap())
            idxsb = res.tile([128, TOTW], I16)
            nc.sync.dma_start(out=idxsb[:], in_=idxT.ap())

            dist2 = res.tile([128, TPC, JTP], F32)
            pm = res.tile([128, TPC, JTP], F32)
            nc.vector.memset(dist2[:], POIS_S)
            nc.vector.memset(pm[:], 0.0)

            selff32 = selfsb[:].bitcast(F32)        # (128, TPC, 64)

            qrr = 0
            off = 0
            for t in range(TPC):
                jt = int(JT[t])
                g = gpool.tile([128, JTP, ROW], F16, tag="g")
                for w in range(NWIN):
                    jm = int(jmax[t, w])
                    if jm == 0:
                        continue
                    jo = int(joff[t, w])
                    nc.gpsimd.dma_gather(
                        out_ap=g[:, jo : jo + jm, :],
                        in_ap=tabT.ap()[w * WSTR : (w + 1) * WSTR, :],
                        idxs_ap=idxsb[:, off + jo * 8 : off + (jo + jm) * 8],
                        num_idxs=jm * 128,
                        num_idxs_reg=jm * 128,
                        elem_size=ROW,
                        single_packet=False,
                        queue_num=qrr,
                    )
                    qrr = (qrr + 1) % NQUEUES
                off += jt * 8

                m = mpool.tile([128, JTP, C], F16, tag="m")
                fb = selfsb[:, t, 0:C].unsqueeze(1).broadcast_to([128, jt, C])
                nc.vector.tensor_tensor(
                    out=m[:, 0:jt, :], in0=g[:, 0:jt, 0:C], in1=fb, op=mybir.AluOpType.mult
                )
                dslice = dist2[:, t, 0:jt]           # (128, jt)
                nc.vector.reduce_sum(
                    out=dslice, in_=m[:, 0:jt, :], axis=mybir.AxisListType.X
                )
                gf32 = g[:].bitcast(F32)             # (128, JTP, 64)
                sj = gf32[:, 0:jt, 32]               # (128, jt)
                nc.vector.scalar_tensor_tensor(
                    out=dslice, in0=dslice, scalar=-2.0, in1=sj,
                    op0=mybir.AluOpType.mult, op1=mybir.AluOpType.add,
                )
                si = selff32[:, t, 32].unsqueeze(1).broadcast_to([128, jt])
                nc.vector.tensor_add(out=dslice, in0=dslice, in1=si)
                nl = g[:, 0:jt, 66]                  # (128, jt)
                li = selfsb[:, t, 66].unsqueeze(1).broadcast_to([128, jt])
                nc.vector.tensor_tensor(
                    out=pm[:, t, 0:jt], in0=nl, in1=li, op=mybir.AluOpType.is_equal,
                )

            # ---- phase 2 ----
            nc.scalar.sqrt(out=dist2[:], in_=dist2[:])
            mind = p2.tile([128, TPC], F32)
            nc.vector.tensor_reduce(
                out=mind[:], in_=dist2[:], axis=mybir.AxisListType.X,
                op=mybir.AluOpType.min,
            )
            mbc = mind[:].unsqueeze(2).broadcast_to([128, TPC, JTP])
            nc.vector.tensor_tensor(
                out=dist2[:], in0=dist2[:], in1=mbc, op=mybir.AluOpType.subtract
            )
            nc.scalar.activation(
                out=dist2[:], in_=dist2[:],
                func=mybir.ActivationFunctionType.Exp, scale=-1.0 / TEMP,
            )
            negs = p2.tile([128, TPC], F32)
            nc.vector.reduce_sum(out=negs[:], in_=dist2[:], axis=mybir.AxisListType.X)
            nc.vector.tensor_tensor(
                out=dist2[:], in0=dist2[:], in1=pm[:], op=mybir.AluOpType.mult
            )
            poss = p2.tile([128, TPC], F32)
            nc.vector.reduce_sum(out=poss[:], in_=dist2[:], axis=mybir.AxisListType.X)
            cnts = p2.tile([128, TPC], F32)
            nc.vector.reduce_sum(out=cnts[:], in_=pm[:], axis=mybir.AxisListType.X)
            rn = p2.tile([128, TPC], F32)
            nc.vector.reciprocal(out=rn[:], in_=negs[:])
            ratio = p2.tile([128, TPC], F32)
            nc.vector.tensor_tensor(
                out=ratio[:], in0=poss[:], in1=rn[:], op=mybir.AluOpType.mult
            )
            eps_t = p2.tile([128, 1], F32)
            nc.vector.memset(eps_t[:], EPS)
            lg = p2.tile([128, TPC], F32)
            nc.scalar.activation(
                out=lg[:], in_=ratio[:],
                func=mybir.ActivationFunctionType.Ln, bias=eps_t[:],
            )
            ma = p2.tile([128, TPC], F32)
            nc.vector.tensor_scalar(
                out=ma[:], in0=cnts[:], scalar1=0.5, scalar2=None,
                op0=mybir.AluOpType.is_gt,
            )
            mb2 = p2.tile([128, TPC], F32)
            nc.vector.tensor_scalar(
                out=mb2[:], in0=cnts[:], scalar1=float(K) - 0.5, scalar2=None,
                op0=mybir.AluOpType.is_lt,
            )
            nc.vector.tensor_tensor(
                out=ma[:], in0=ma[:], in1=mb2[:], op=mybir.AluOpType.mult
            )
            nc.vector.tensor_tensor(
                out=ma[:], in0=ma[:], in1=validsb[:], op=mybir.AluOpType.mult
            )
            nc.vector.tensor_tensor(
                out=lg[:], in0=lg[:], in1=ma[:], op=mybir.AluOpType.mult
            )
            outsb = p2.tile([128, 2], F32)
            nc.vector.reduce_sum(out=outsb[:, 0:1], in_=lg[:], axis=mybir.AxisListType.X)
            nc.vector.reduce_sum(out=outsb[:, 1:2], in_=ma[:], axis=mybir.AxisListType.X)
            nc.sync.dma_start(out=outT.ap(), in_=outsb[:])

    nc.compile()
    return nc


def run(features, labels, neighbor_idx, trace=False):
    packed, table = _pack_table(features, labels)
    plan = _build_plan(neighbor_idx)
    jmax, joff, JT, JTP = plan["jmax"], plan["joff"], plan["JT"], plan["JTP"]
    TOTW = int(JT.sum()) * 8

    key = (JTP, TOTW, jmax.tobytes())
    if _CACHE.get("key") != key:
        _CACHE["nc"] = _build(jmax, joff, JT, JTP, TOTW)
        _CACHE["key"] = key
    nc = _CACHE["nc"]

    in_maps = []
    for c in range(NCORES):
        m = _core_inputs(plan, packed, c)
        m["tab"] = table
        in_maps.append(m)
    res = bass_utils.run_bass_kernel_spmd(
        nc, in_maps, core_ids=list(range(NCORES)), trace=trace
    )
    s = 0.0
    ccnt = 0.0
    for o in res.results:
        s += float(o["out"][:, 0].astype(np.float64).sum())
        ccnt += float(o["out"][:, 1].astype(np.float64).sum())
    loss = np.float32(-s / max(ccnt, 1.0))
    return loss, res


def kernel(features, labels, neighbor_idx):
    loss, _ = run(features, labels, neighbor_idx, trace=False)
    return loss


# revision 10
# speedup vs baseline: 8.7523x; 1.0088x over previous
"""Trainium2 Bass kernel for ContrastHead (softnn contrastive KNN loss).

Data-parallel over points on 8 cores. Host packs a table row per point
(256B = 128 f16 slots): [f16 features(64) | f32 ||f||^2 | f16 label | pad].
Table = 4 windows of 32766 rows, each prefixed by a POISON row (features 0,
||f||^2 = 1e6, label -1) so int16 dma_gather indices address any row.
Points are globally sorted by their per-window neighbor-count profile and
dealt round-robin to cores, so all cores' tile-t lane profiles are nearly
identical and the shared per-tile per-window block sizes stay tight. Each
point's 31 neighbors are sorted by (window, index). Per tile, one gather per
window writes a dense j-slice of a single (128, JT_t, 256B) buffer — no
dummy-row traffic (~3.3x fewer descriptors than the fixed 4-window scheme).
Ragged slots gather the poison row, which self-masks: exp((min-1000)/T)=0
and label -1 never matches. Gathers round-robin the 4 SWDGE queues so
descriptor generation runs on all four Q7 core pairs concurrently; all
indices are preloaded to SBUF so gathers never wait on index DMAs.
Per tile: DVE f16 multiply + tree-add + reduce -> dots; dist^2 =
s_i + s_j - 2*dot; posmask via is_equal. Phase 2 on (128, TPC*JTP):
sqrt -> row min -> exp((min-d)/T) -> neg/pos sums -> ratio -> Ln(+1e-8) ->
point mask -> (128, 2) accumulators. Host sums 8x(128,2).
"""

import numpy as np

import concourse.bacc as bacc
import concourse.bass as bass
import concourse.mybir as mybir
import concourse.tile as tile
from concourse import bass_utils

F16 = mybir.dt.float16
F32 = mybir.dt.float32
I16 = mybir.dt.int16

N = 100000
K = 31
C = 64
ROW = 128                   # f16 slots per table row (256B)
WINR = 32766                # real rows per window
WSTR = WINR + 1             # window stride in table (incl poison row 0)
NWIN = 4
TABROWS = NWIN * WSTR
NCORES = 8
PTS = N // NCORES           # 12500
TPC = (PTS + 127) // 128    # 98 tiles/core
TEMP = 0.1
EPS = 1e-8
POIS_S = 1.0e6
NQUEUES = 4

_CACHE = {}


# ---------------- host-side plan + packing ----------------

def _pack_table(features, labels):
    packed = np.zeros((N, ROW), dtype=np.float16)
    packed[:, 0:C] = features.astype(np.float16)
    s = np.sum(features.astype(np.float64) ** 2, axis=1).astype(np.float32)
    packed[:, 64:66] = s[:, None].view(np.float16)
    packed[:, 66] = labels.astype(np.float16)

    tab = np.zeros((TABROWS, ROW), dtype=np.float16)
    pr = np.zeros((ROW,), dtype=np.float16)
    pr[64:66] = np.array([POIS_S], dtype=np.float32).view(np.float16)
    pr[66] = -1.0
    for w in range(NWIN):
        lo = w * WINR
        hi = min(lo + WINR, N)
        tab[w * WSTR] = pr
        if hi > lo:
            tab[w * WSTR + 1 : w * WSTR + 1 + (hi - lo)] = packed[lo:hi]
    return packed, tab


def _build_plan(neighbor_idx):
    """Global profile-sorted round-robin sharding + shared gather plan."""
    w_all = (neighbor_idx // WINR).astype(np.int32)
    loc_all = (neighbor_idx - w_all * WINR + 1).astype(np.int32)

    cnt = np.stack([(w_all == j).sum(1) for j in range(NWIN)], axis=1).astype(np.int32)
    order = np.lexsort((cnt[:, 3], cnt[:, 1], cnt[:, 0]))          # (N,) global
    # neighbors of each point sorted by (window, index) for HBM locality
    ks = np.argsort(w_all * 32768 + loc_all, axis=1)
    loc_s = np.take_along_axis(loc_all, ks, axis=1)
    cum = np.zeros((N, NWIN), np.int32)
    cum[:, 1:] = np.cumsum(cnt, axis=1)[:, :-1]

    # shared per-tile blocks: tile t holds sorted ranks [t*1024, (t+1)*1024)
    padN = TPC * 128 * NCORES
    cnt_p = np.zeros((padN, NWIN), np.int32)
    cnt_p[:N] = cnt[order]
    jmax = cnt_p.reshape(TPC, 128 * NCORES, NWIN).max(axis=1)       # (TPC, NWIN)
    joff = np.zeros((TPC, NWIN), np.int32)
    joff[:, 1:] = np.cumsum(jmax, axis=1)[:, :-1]
    JT = jmax.sum(axis=1)
    return dict(jmax=jmax, joff=joff, JT=JT, JTP=int(JT.max()),
                order=order, cnt=cnt, cum=cum, loc_s=loc_s)


def _core_inputs(plan, packed, c):
    order, cnt, cum, loc_s = plan["order"], plan["cnt"], plan["cum"], plan["loc_s"]
    jmax, JT = plan["jmax"], plan["JT"]
    # core c, slot q (= t*128+p) <- global sorted rank q*8+c
    ranks = np.arange(TPC * 128) * NCORES + c
    real = ranks < N
    pts = np.where(real, order[np.minimum(ranks, N - 1)], 0)        # (TPC*128,)

    segs = []
    for t in range(TPC):
        pts_t = pts[t * 128 : (t + 1) * 128]
        real_t = real[t * 128 : (t + 1) * 128]
        tile_flat = []
        for w in range(NWIN):
            jm = int(jmax[t, w])
            if jm == 0:
                continue
            n_w = np.where(real_t, cnt[pts_t, w], 0)
            st = cum[pts_t, w]
            col = st[:, None] + np.arange(jm)[None, :]
            validm = np.arange(jm)[None, :] < n_w[:, None]
            vals = np.where(
                validm,
                np.take_along_axis(loc_s[pts_t], np.minimum(col, K - 1), axis=1),
                0,
            )
            tile_flat.append(vals.T.reshape(-1))     # j-major flat
        flat = np.concatenate(tile_flat).astype(np.int16)
        wrapped = flat.reshape(-1, 16).T             # (16, JT_t*8)
        segs.append(np.tile(wrapped, (8, 1)))        # (128, JT_t*8)
    idx16 = np.ascontiguousarray(np.concatenate(segs, axis=1))

    rows = np.where(real[:, None], packed[pts], 0).astype(np.float16)
    selftab = np.ascontiguousarray(rows.reshape(TPC, 128, ROW).transpose(1, 0, 2))
    valid = real.astype(np.float32)
    valid = np.ascontiguousarray(valid.reshape(TPC, 128).transpose(1, 0))
    return dict(selftab=selftab, nidx16=idx16, valid=valid)


# ---------------- device program ----------------

def _build(jmax, joff, JT, JTP, TOTW):
    nc = bacc.Bacc(
        "TRN2", target_bir_lowering=False, debug=False, num_swdge_queues=NQUEUES
    )

    tabT = nc.dram_tensor("tab", (TABROWS, ROW), F16, kind="ExternalInput")
    selfT = nc.dram_tensor("selftab", (128, TPC, ROW), F16, kind="ExternalInput")
    idxT = nc.dram_tensor("nidx16", (128, TOTW), I16, kind="ExternalInput")
    validT = nc.dram_tensor("valid", (128, TPC), F32, kind="ExternalInput")
    outT = nc.dram_tensor("out", (128, 2), F32, kind="ExternalOutput")

    with tile.TileContext(nc) as tc:
        with (
            tc.tile_pool(name="res", bufs=1) as res,
            tc.tile_pool(name="gpool", bufs=5) as gpool,
            tc.tile_pool(name="p2", bufs=1) as p2,
        ):
            selfsb = res.tile([128, TPC, ROW], F16)
            nc.sync.dma_start(out=selfsb[:], in_=selfT.ap())
            validsb = res.tile([128, TPC], F32)
            nc.sync.dma_start(out=validsb[:], in_=validT.ap())
            idxsb = res.tile([128, TOTW], I16)
            nc.sync.dma_start(out=idxsb[:], in_=idxT.ap())

            dist2 = res.tile([128, TPC, JTP], F32)
            pm = res.tile([128, TPC, JTP], F32)
            nc.vector.memset(dist2[:], POIS_S)
            nc.vector.memset(pm[:], 0.0)

            selff32 = selfsb[:].bitcast(F32)        # (128, TPC, 64)

            qrr = 0
            off = 0
            for t in range(TPC):
                jt = int(JT[t])
                g = gpool.tile([128, JTP, ROW], F16, tag="g")
                for w in range(NWIN):
                    jm = int(jmax[t, w])
                    if jm == 0:
                        continue
                    jo = int(joff[t, w])
                    nc.gpsimd.dma_gather(
                        out_ap=g[:, jo : jo + jm, :],
                        in_ap=tabT.ap()[w * WSTR : (w + 1) * WSTR, :],
                        idxs_ap=idxsb[:, off + jo * 8 : off + (jo + jm) * 8],
                        num_idxs=jm * 128,
                        num_idxs_reg=jm * 128,
                        elem_size=ROW,
                        single_packet=False,
                        queue_num=qrr,
                    )
                    qrr = (qrr + 1) % NQUEUES
                off += jt * 8

                fb = selfsb[:, t, 0:C].unsqueeze(1).broadcast_to([128, jt, C])
                # in-place: overwrite gathered features with elementwise products
                # (slots 64:67 = ||f||^2 / label stay intact)
                nc.vector.tensor_tensor(
                    out=g[:, 0:jt, 0:C], in0=g[:, 0:jt, 0:C], in1=fb,
                    op=mybir.AluOpType.mult,
                )
                dslice = dist2[:, t, 0:jt]           # (128, jt)
                nc.vector.reduce_sum(
                    out=dslice, in_=g[:, 0:jt, 0:C], axis=mybir.AxisListType.X
                )
                gf32 = g[:].bitcast(F32)             # (128, JTP, 64)
                sj = gf32[:, 0:jt, 32]               # (128, jt)
                nc.vector.scalar_tensor_tensor(
                    out=dslice, in0=dslice, scalar=-2.0, in1=sj,
                    op0=mybir.AluOpType.mult, op1=mybir.AluOpType.add,
                )
                si = selff32[:, t, 32].unsqueeze(1).broadcast_to([128, jt])
                nc.vector.tensor_add(out=dslice, in0=dslice, in1=si)
                nl = g[:, 0:jt, 66]                  # (128, jt)
                li = selfsb[:, t, 66].unsqueeze(1).broadcast_to([128, jt])
                nc.vector.tensor_tensor(
                    out=pm[:, t, 0:jt], in0=nl, in1=li, op=mybir.AluOpType.is_equal,
                )

            # ---- phase 2 ----
            nc.scalar.sqrt(out=dist2[:], in_=dist2[:])
            mind = p2.tile([128, TPC], F32)
            nc.vector.tensor_reduce(
                out=mind[:], in_=dist2[:], axis=mybir.AxisListType.X,
                op=mybir.AluOpType.min,
            )
            mbc = mind[:].unsqueeze(2).broadcast_to([128, TPC, JTP])
            nc.vector.tensor_tensor(
                out=dist2[:], in0=dist2[:], in1=mbc, op=mybir.AluOpType.subtract
            )
            nc.scalar.activation(
                out=dist2[:], in_=dist2[:],
                func=mybir.ActivationFunctionType.Exp, scale=-1.0 / TEMP,
            )
            negs = p2.tile([128, TPC], F32)
            nc.vector.reduce_sum(out=negs[:], in_=dist2[:], axis=mybir.AxisListType.X)
            nc.vector.tensor_tensor(
                out=dist2[:], in0=dist2[:], in1=pm[:], op=mybir.AluOpType.mult
            )
            poss = p2.tile([128, TPC], F32)
            nc.vector.reduce_sum(out=poss[:], in_=dist2[:], axis=mybir.AxisListType.X)
            cnts = p2.tile([128, TPC], F32)
            nc.vector.reduce_sum(out=cnts[:], in_=pm[:], axis=mybir.AxisListType.X)
            rn = p2.tile([128, TPC], F32)
            nc.vector.reciprocal(out=rn[:], in_=negs[:])
            ratio = p2.tile([128, TPC], F32)
            nc.vector.tensor_tensor(
                out=ratio[:], in0=poss[:], in1=rn[:], op=mybir.AluOpType.mult
            )
            eps_t = p2.tile([128, 1], F32)
            nc.vector.memset(eps_t[:], EPS)
            lg = p2.tile([128, TPC], F32)
            nc.scalar.activation(
                out=lg[:], in_=ratio[:],
                func=mybir.ActivationFunctionType.Ln, bias=eps_t[:],
            )
            ma = p2.tile([128, TPC], F32)
            nc.vector.tensor_scalar(
                out=ma[:], in0=cnts[:], scalar1=0.5, scalar2=None,
                op0=mybir.AluOpType.is_gt,
            )
            mb2 = p2.tile([128, TPC], F32)
            nc.vector.tensor_scalar(
                out=mb2[:], in0=cnts[:], scalar1=float(K) - 0.5, scalar2=None,
                op0=mybir.AluOpType.is_lt,
            )
            nc.vector.tensor_tensor(
                out=ma[:], in0=ma[:], in1=mb2[:], op=mybir.AluOpType.mult
            )
            nc.vector.tensor_tensor(
                out=ma[:], in0=ma[:], in1=validsb[:], op=mybir.AluOpType.mult
            )
            nc.vector.tensor_tensor(
                out=lg[:], in0=lg[:], in1=ma[:], op=mybir.AluOpType.mult
            )
            outsb = p2.tile([128, 2], F32)
            nc.vector.reduce_sum(out=outsb[:, 0:1], in_=lg[:], axis=mybir.AxisListType.X)
            nc.vector.reduce_sum(out=outsb[:, 1:2], in_=ma[:], axis=mybir.AxisListType.X)
            nc.sync.dma_start(out=outT.ap(), in_=outsb[:])

    nc.compile()
    return nc


def run(features, labels, neighbor_idx, trace=False):
    packed, table = _pack_table(features, labels)
    plan = _build_plan(neighbor_idx)
    jmax, joff, JT, JTP = plan["jmax"], plan["joff"], plan["JT"], plan["JTP"]
    TOTW = int(JT.sum()) * 8

    key = (JTP, TOTW, jmax.tobytes())
    if _CACHE.get("key") != key:
        _CACHE["nc"] = _build(jmax, joff, JT, JTP, TOTW)
        _CACHE["key"] = key
    nc = _CACHE["nc"]

    in_maps = []
    for c in range(NCORES):
        m = _core_inputs(plan, packed, c)
        m["tab"] = table
        in_maps.append(m)
    res = bass_utils.run_bass_kernel_spmd(
        nc, in_maps, core_ids=list(range(NCORES)), trace=trace
    )
    s = 0.0
    ccnt = 0.0
    for o in res.results:
        s += float(o["out"][:, 0].astype(np.float64).sum())
        ccnt += float(o["out"][:, 1].astype(np.float64).sum())
    loss = np.float32(-s / max(ccnt, 1.0))
    return loss, res


def kernel(features, labels, neighbor_idx):
    loss, _ = run(features, labels, neighbor_idx, trace=False)
    return loss
